# revision 9
# baseline (speedup 1.0000x reference)
"""Bass/Trainium2 kernel for nn_DDSOpWithReductionOpModel.

Computes out = nonzero(x).sum(dim=0) for x [8192, 8192] fp32 -> [2] int64:
  out[0] = sum of row indices of nonzero elements
  out[1] = sum of col indices of nonzero elements

Equivalently, with per-row counts r[i] and per-column counts c[j] of nonzero
elements: out[0] = dot(arange(8192), r), out[1] = dot(arange(8192), c).

Strategy (data-parallel over 8 NeuronCores, rows sharded 1024/core):
  On device, per core (local shard x [1024, 8192]):
    - stream 8 tiles of [128, 8192] from HBM
    - one DVE tensor_scalar pass per tile: mask = (x != 0) as bf16, with
      accum_out giving the per-partition (= per-row) nonzero count
    - column counts via PE: ones[128,1].T @ mask[:, 512-chunk] accumulated in
      PSUM across the 8 row tiles (16 chunks live at 4 partition offsets x 4
      PSUM banks)
  Counts are integers <= 8192, exact in fp32/PSUM. Host does the tiny exact
  int64 dot with arange and the 8-way reduction of column counts.
"""

import numpy as np

import concourse.bacc as bacc
import concourse.mybir as mybir
from concourse.bass_utils import run_bass_kernel_spmd
from concourse.tile import TileContext

N0, N1 = 8192, 8192
N_CORES = 8
R = N0 // N_CORES  # rows per core


def build_nc(rows=R, cols=N1, x_bufs=3, mask_bufs=2):
    """Build the per-core Bass module (SPMD: every core runs this program on
    its own [rows, cols] shard)."""
    assert rows % 128 == 0 and cols % 128 == 0
    nt = rows // 128
    n_chunks = cols // 128  # column chunks of 128, one PSUM column each
    assert n_chunks * 4 <= 2048  # fits one PSUM bank (2KB/partition)

    nc = bacc.Bacc("TRN2", target_bir_lowering=False)
    x = nc.dram_tensor("x", [rows, cols], mybir.dt.float32, kind="ExternalInput")
    row_cnt = nc.dram_tensor(
        "row_cnt", [128, nt], mybir.dt.float32, kind="ExternalOutput"
    )
    col_cnt = nc.dram_tensor(
        "col_cnt", [128, n_chunks], mybir.dt.float32, kind="ExternalOutput"
    )

    with TileContext(nc) as tc:
        with (
            tc.tile_pool(name="xp", bufs=x_bufs) as xp,
            tc.tile_pool(name="mp", bufs=mask_bufs) as mp,
            tc.tile_pool(name="pp", bufs=1, space="PSUM") as pp,
            tc.tile_pool(name="cp", bufs=1) as cp,
        ):
            ones = cp.tile([128, 1], mybir.dt.bfloat16)
            nc.vector.memset(ones, 1.0)
            rc = cp.tile([128, nt], mybir.dt.float32)
            psum_col = pp.tile([128, n_chunks], mybir.dt.float32)
            for t in range(nt):
                xt = xp.tile([128, cols], mybir.dt.float32)
                nc.gpsimd.dma_start(out=xt, in_=x[t * 128 : (t + 1) * 128, :])
                mt = mp.tile([128, cols], mybir.dt.bfloat16)
                # mask = (x != 0); accum_out = per-row count of this tile
                nc.vector.tensor_scalar(
                    out=mt,
                    in0=xt,
                    scalar1=0.0,
                    scalar2=None,
                    op0=mybir.AluOpType.not_equal,
                    op1=mybir.AluOpType.add,
                    accum_out=rc[:, t : t + 1],
                )
                # column partial sums: mask chunk [128, 128] as stationary
                # weights, ones [128, 1] streaming -> out [128, 1] = per-column
                # counts of this chunk; PSUM-accumulate over row tiles
                # whole PSUM bank is one accumulation group: start marks the
                # full 2KB zero region pending-zero, later writes accumulate
                for c in range(n_chunks):
                    nc.tensor.matmul(
                        psum_col[:, c : c + 1],
                        lhsT=mt[:, c * 128 : (c + 1) * 128],
                        rhs=ones,
                        start=(t == 0 and c == 0),
                        stop=(t == nt - 1 and c == n_chunks - 1),
                    )
            nc.gpsimd.dma_start(out=row_cnt.ap(), in_=rc)
            col_sb = cp.tile([128, n_chunks], mybir.dt.float32)
            nc.vector.tensor_copy(out=col_sb, in_=psum_col)
            nc.gpsimd.dma_start(out=col_cnt.ap(), in_=col_sb)
    nc.compile()
    return nc


_NC_CACHE = {}


def _get_nc():
    if "nc" not in _NC_CACHE:
        _NC_CACHE["nc"] = build_nc()
    return _NC_CACHE["nc"]


def postprocess(results, rows=R, cols=N1):
    """Combine per-core row/col counts into the [2] int64 output."""
    nt = rows // 128
    out_rows = np.int64(0)
    col_counts = np.zeros(cols, dtype=np.int64)
    for core, res in enumerate(results):
        rc = np.rint(np.asarray(res["row_cnt"], dtype=np.float64)).astype(np.int64)
        # rc[p, t] = count for local row t*128 + p
        local = rc.T.reshape(rows)
        row_idx = np.arange(core * rows, (core + 1) * rows, dtype=np.int64)
        out_rows += np.dot(row_idx, local)
        # cc[p, c] = count for column c*128 + p
        cc = np.rint(np.asarray(res["col_cnt"], dtype=np.float64)).astype(np.int64)
        col_counts += cc.T.reshape(cols)
    out_cols = np.dot(np.arange(cols, dtype=np.int64), col_counts)
    return np.array([out_rows, out_cols], dtype=np.int64)


def kernel(inputs, _trace=False, _trace_kwargs=None):
    x = np.ascontiguousarray(np.asarray(inputs, dtype=np.float32))
    assert x.shape == (N0, N1)
    in_maps = [{"x": x[c * R : (c + 1) * R]} for c in range(N_CORES)]
    res = run_bass_kernel_spmd(
        _get_nc(),
        in_maps,
        core_ids=list(range(N_CORES)),
        trace=_trace,
        **(_trace_kwargs or {}),
    )
    out = postprocess(res.results)
    if _trace:
        return out, res
    return out
